# revision 31
# baseline (speedup 1.0000x reference)
"""Sliding-window softcapped GQA attention, tensor-parallel across 8 NeuronCores.

Sharding (per spec hint): core c owns KV head c and Q heads 4c..4c+3.
Each core computes x->q/k/v proj, QK-RMSNorm, RoPE, windowed softcapped
attention, and its partial o_proj; host sums the 8 partial outputs.

v2: single fused pass per 512-token block. o_proj of block i-1 is emitted as
PE filler inside the norm/attention phases of block i (score-ahead pipelining
keeps the PE dense while the Act engine runs tanh/exp). Softmax/RMSNorm
reciprocals use the fast DVE approximation, partition broadcasts run on
GpSimd, the RoPE half-swap is a DVE stream_shuffle, and V is transposed by
the DMA XBAR. Weights and all latency-critical state stay resident in SBUF.
"""
import numpy as np

B, S, HID = 2, 2048, 4096
NQ, NK, HD = 32, 8, 128
WINDOW = 1024
SOFTCAP = 50.0
EPS = 1e-6
NCORES = 8
QD = NQ // NCORES * HD      # 512 q-dims per core
TOK = B * S                 # 4096 tokens
NBLK = 4                    # q-blocks of 512 per batch
BLK = 512
KTILES = HID // 128         # 32 k tiles over hidden
NH = NQ // NCORES           # 4 q heads per core

_CACHE = {}


def _window_jts(qb):
    lo = max(0, qb * BLK - (WINDOW - 1)) // 128
    hi = (qb * BLK + BLK - 1) // 128
    return lo, hi


def _tile_mask_kind(qb, jt):
    """None = fully allowed, 'causal' or 'window' = needs affine mask."""
    if jt * 128 + 127 > qb * BLK:
        return "causal"
    if jt * 128 < qb * BLK - BLK:
        return "window"
    return None


def _build(_debug=False):
    import concourse.bass as bass
    import concourse.mybir as mybir
    import concourse.tile as tile
    from concourse import bacc
    from concourse.masks import make_identity

    f32, f32r, bf16 = mybir.dt.float32, mybir.dt.float32r, mybir.dt.bfloat16
    AF = mybir.ActivationFunctionType
    ALU = mybir.AluOpType

    nc = bacc.Bacc("TRN2", target_bir_lowering=False, debug=False,
                   num_devices=NCORES)

    # ---- DRAM I/O (per-core shapes; same program on all cores) ----
    xT = nc.dram_tensor("xT", (HID, TOK), bf16, kind="ExternalInput").ap()
    wqT = nc.dram_tensor("wqT", (128, KTILES * QD), bf16,
                         kind="ExternalInput").ap()
    wkT = nc.dram_tensor("wkT", (128, KTILES * 128), bf16,
                         kind="ExternalInput").ap()
    wvT = nc.dram_tensor("wvT", (128, KTILES * 128), bf16,
                         kind="ExternalInput").ap()
    woT = nc.dram_tensor("woT", (128, NH * HID), bf16,
                         kind="ExternalInput").ap()
    cosqT = nc.dram_tensor("cosqT", (128, S), bf16, kind="ExternalInput").ap()
    sinqT = nc.dram_tensor("sinqT", (128, S), bf16, kind="ExternalInput").ap()
    coskT = nc.dram_tensor("coskT", (128, S), bf16, kind="ExternalInput").ap()
    sinkT = nc.dram_tensor("sinkT", (128, S), bf16, kind="ExternalInput").ap()
    outT = nc.dram_tensor("outT", (HID, TOK), bf16, kind="ExternalOutput").ap()
    if _debug:
        dbg_khat = nc.dram_tensor("dbg_khat", (128, S), f32,
                                  kind="ExternalOutput").ap()
        dbg_vnat = nc.dram_tensor("dbg_vnat", (128, S), bf16,
                                  kind="ExternalOutput").ap()
        dbg_qhat = nc.dram_tensor("dbg_qhat", (128, NH * BLK), f32,
                                  kind="ExternalOutput").ap()

    with tile.TileContext(nc) as tc:
        with tc.tile_pool(name="wts", bufs=1) as wts, \
             tc.tile_pool(name="stream", bufs=2) as stream, \
             tc.tile_pool(name="persist", bufs=1) as persist, \
             tc.tile_pool(name="work", bufs=2) as work, \
             tc.tile_pool(name="ps", bufs=1, space="PSUM") as ps:

            # ---- resident constants / weights ----
            wq_s = wts.tile([128, KTILES * QD], bf16)       # 32KB/p
            wk_s = wts.tile([128, KTILES * 128], bf16)      # 8KB/p
            wv_s = wts.tile([128, KTILES * 128], bf16)      # 8KB/p
            wo_s = wts.tile([128, NH * HID], bf16)          # 32KB/p
            for k in range(0, KTILES, 8):
                nc.sync.dma_start(wq_s[:, k * QD:(k + 8) * QD],
                                  wqT[:, k * QD:(k + 8) * QD])
            nc.sync.dma_start(wk_s[:], wkT[:])
            nc.sync.dma_start(wv_s[:], wvT[:])
            nc.sync.dma_start(wo_s[:], woT[:])
            cosq = wts.tile([128, S], bf16)
            sinq = wts.tile([128, S], bf16)
            cosk = wts.tile([128, S], bf16)
            sink = wts.tile([128, S], bf16)
            nc.sync.dma_start(cosq[:], cosqT[:])
            nc.sync.dma_start(sinq[:], sinqT[:])
            nc.sync.dma_start(cosk[:], coskT[:])
            nc.sync.dma_start(sink[:], sinkT[:])

            ones_b = wts.tile([128, 1], bf16)               # colsum lhsT
            nc.gpsimd.memset(ones_b[:], 1.0)
            onesrow_f = wts.tile([1, 128], f32)
            nc.gpsimd.memset(onesrow_f[:], 1.0)
            onesrow_r = wts.tile([1, 128], f32r)            # bcast lhsT
            nc.vector.tensor_copy(onesrow_r[:], onesrow_f[:])
            neg50 = wts.tile([128, 1], f32)
            nc.gpsimd.memset(neg50[:], -50.0)
            ident_f = wts.tile([128, 128], f32)
            make_identity(nc, ident_f[:])
            ident_b = wts.tile([128, 128], bf16)
            nc.vector.tensor_copy(ident_b[:], ident_f[:])
            # half-swap permutation: swap[i, j] = 1 iff j == (i+64) % 128
            swap_f = wts.tile([128, 128], f32)
            nc.gpsimd.memset(swap_f[:], 0.0)
            nc.gpsimd.affine_select(out=swap_f[:], in_=swap_f[:],
                                    compare_op=ALU.not_equal, fill=1.0,
                                    base=64, pattern=[[-1, 128]],
                                    channel_multiplier=1)
            nc.gpsimd.affine_select(out=swap_f[:], in_=swap_f[:],
                                    compare_op=ALU.not_equal, fill=1.0,
                                    base=-64, pattern=[[-1, 128]],
                                    channel_multiplier=1)
            swap_r = wts.tile([128, 128], f32r)
            nc.vector.tensor_copy(swap_r[:], swap_f[:])

            # persistent per-batch state (overwritten per batch)
            khat = persist.tile([128, S], f32r, tag="khat")
            vnat = persist.tile([128, S], bf16, tag="vnat")

            # ---------------- emission helpers ----------------

            def qkv_phase(b, qb):
                tok0 = b * S + qb * BLK
                qp01 = ps.tile([128, 2 * BLK], f32, tag="uni", bufs=3)
                qp23 = ps.tile([128, 2 * BLK], f32, tag="uni", bufs=3)
                kvp = ps.tile([128, 2 * BLK], f32, tag="uni", bufs=3)
                qsl = [qp01[:, 0:BLK], qp01[:, BLK:2 * BLK],
                       qp23[:, 0:BLK], qp23[:, BLK:2 * BLK]]
                ksl, vsl = kvp[:, 0:BLK], kvp[:, BLK:2 * BLK]
                for kk in range(KTILES):
                    sl = stream.tile([128, BLK], bf16, tag="slab", bufs=4)
                    nc.sync.dma_start(sl[:],
                                      xT[kk * 128:(kk + 1) * 128,
                                         tok0:tok0 + BLK])
                    if True:
                        xt = sl[:]
                        for m in range(NH):
                            nc.tensor.matmul(
                                qsl[m],
                                wq_s[:, kk * QD + m * 128:kk * QD + (m + 1) * 128],
                                xt, start=(kk == 0), stop=(kk == KTILES - 1))
                        nc.tensor.matmul(ksl, wk_s[:, kk * 128:(kk + 1) * 128],
                                         xt, start=(kk == 0), stop=(kk == KTILES - 1))
                        nc.tensor.matmul(vsl, wv_s[:, kk * 128:(kk + 1) * 128],
                                         xt, start=(kk == 0), stop=(kk == KTILES - 1))
                return qp01, qp23, kvp

            def norm_phase(b, qb, qp01, qp23, kvp, qhat, filler):
                """QK-RMSNorm + RoPE for 4 q heads + k; V -> vnat via PE.
                Copies the qkv psums to SBUF immediately so the uni psum
                slots free up for o_proj filler items of the prev block."""
                pos0 = qb * BLK
                # squares read the psums (Act), then the psums are copied to
                # SBUF (DVE) and released for the uni rotation.
                sq01 = work.tile([128, 2 * BLK], bf16, tag="sq", bufs=1)
                nc.scalar.activation(sq01[:], qp01[:], AF.Square)
                sq23 = work.tile([128, 2 * BLK], bf16, tag="sq2", bufs=1)
                nc.scalar.activation(sq23[:], qp23[:], AF.Square)
                sqk = work.tile([128, BLK], bf16, tag="sqk", bufs=1)
                nc.scalar.activation(sqk[:], kvp[:, 0:BLK], AF.Square)
                qraw = work.tile([128, 5 * BLK], f32r, tag="qraw", bufs=1)
                nc.vector.tensor_copy(qraw[:, 0:2 * BLK], qp01[:])
                nc.vector.tensor_copy(qraw[:, 2 * BLK:4 * BLK], qp23[:])
                nc.vector.tensor_copy(qraw[:, 4 * BLK:5 * BLK], kvp[:, 0:BLK])
                vT_s = work.tile([128, BLK], bf16, tag="vTs", bufs=1)
                nc.vector.tensor_copy(vT_s[:], kvp[:, BLK:2 * BLK])
                # V transpose to natural layout (PE, via acc rotation)
                vtr = ps.tile([128, BLK], bf16, tag="acc", bufs=1)
                for tt in range(4):
                    nc.tensor.transpose(vtr[:, tt * 128:(tt + 1) * 128],
                                        vT_s[:, tt * 128:(tt + 1) * 128],
                                        ident_b[:])
                nc.vector.tensor_copy(vnat[:, qb * BLK:(qb + 1) * BLK], vtr[:])
                for _ in range(3):
                    if filler:
                        filler.pop(0)()
                # column sums + rsqrt (folded scales):
                #   q: sqrt((1/ss)/2500) = rsqrt(ss)/50
                #   k: sqrt((1/ss)*HD)   = rsqrt(mean(k^2))
                sqs = [sq01[:, 0:BLK], sq01[:, BLK:2 * BLK],
                       sq23[:, 0:BLK], sq23[:, BLK:2 * BLK], sqk[:]]
                nrms = []
                for m in (0, 4, 1, 2, 3):
                    cst = ps.tile([1, BLK], f32, tag="acc", bufs=1)
                    nc.tensor.matmul(cst[:], ones_b[:], sqs[m],
                                     start=True, stop=True)
                    rq = work.tile([1, BLK], f32, tag="rq", bufs=2)
                    nc.vector.reciprocal_approx_fast(rq[:], cst[:])
                    rs2 = work.tile([1, BLK], f32r, tag="rs2", bufs=2)
                    scale = float(HD) if m == 4 else 1.0 / (SOFTCAP * SOFTCAP)
                    with nc.allow_low_precision(reason="f32r bcast rhs"):
                        nc.scalar.activation(rs2[:], rq[:], AF.Sqrt, scale=scale)
                    bcq = ps.tile([128, BLK], f32, tag="acc", bufs=1)
                    nc.tensor.matmul(bcq[:], onesrow_r[:],
                                     rs2[:], start=True, stop=True)
                    nrm = work.tile([128, BLK], f32r, tag="nrm", bufs=5)
                    nc.vector.tensor_mul(nrm[:],
                                         qraw[:, m * BLK:(m + 1) * BLK], bcq[:])
                    nrms.append((m, nrm))
                # rope per head (q0 first, then k, then q1..q3)
                for m, nrm in nrms:
                    rot = ps.tile([128, BLK], f32, tag="acc", bufs=1)
                    nc.tensor.matmul(rot[:], swap_r[:], nrm[:],
                                     start=True, stop=True)
                    if m < 4:
                        dst = qhat[:, m * BLK:(m + 1) * BLK]
                        cs_t, sn_t = cosq, sinq
                    else:
                        dst = khat[:, qb * BLK:(qb + 1) * BLK]
                        cs_t, sn_t = cosk, sink
                    nc.vector.tensor_mul(dst, nrm[:],
                                         cs_t[:, pos0:pos0 + BLK])
                    m2 = work.tile([128, BLK], f32, tag="m2", bufs=2)
                    nc.vector.tensor_mul(m2[:], rot[:],
                                         sn_t[:, pos0:pos0 + BLK])
                    nc.vector.tensor_add(dst, dst, m2[:])
                    if m in (4, 1) and filler:
                        filler.pop(0)()

            def oproj_items(oth_tiles, tok0):
                """o_proj work items for one finished block: 16 psum tiles,
                each covering two 128-row output feature tiles."""
                items = []
                for mp in range(16):
                    def item(m0=2 * mp):
                        op = ps.tile([128, 2 * BLK], f32, tag="uni", bufs=3)
                        for half in range(2):
                            m = m0 + half
                            for kk in range(NH):
                                nc.tensor.matmul(
                                    op[:, half * BLK:(half + 1) * BLK],
                                    wo_s[:, kk * HID + m * 128:kk * HID + (m + 1) * 128],
                                    oth_tiles[kk][:],
                                    start=(kk == 0), stop=(kk == NH - 1))
                        for half in range(2):
                            og = work.tile([128, BLK], bf16, tag="og", bufs=2)
                            nc.vector.tensor_copy(
                                og[:], op[:, half * BLK:(half + 1) * BLK])
                            m = m0 + half
                            nc.sync.dma_start(
                                outT[m * 128:(m + 1) * 128, tok0:tok0 + BLK],
                                og[:])
                    items.append(item)
                return items

            def attn_phase(b, qb, qhat, filler):
                """Attention for 4 heads with score-ahead pipelining; PE gaps
                are filled with o_proj items of the previous block."""
                lo, hi = _window_jts(qb)
                oth_tiles = []

                def scores(qh, jp):
                    sp2 = ps.tile([128, 2 * BLK], f32, tag="uni", bufs=3)
                    for half, jt in enumerate((jp, jp + 1)):
                        nc.tensor.matmul(
                            sp2[:, half * BLK:(half + 1) * BLK],
                            khat[:, jt * 128:(jt + 1) * 128],
                            qh, start=True, stop=True)
                    return sp2

                def act_chain(sp2, jp):
                    # softcap tanh in place on psum, exp to bf16, mask after
                    nc.scalar.activation(sp2[:], sp2[:], AF.Tanh)
                    pt = work.tile([128, 2 * BLK], bf16, tag="pt", bufs=2)
                    nc.scalar.activation(pt[:], sp2[:], AF.Exp,
                                         scale=SOFTCAP, bias=neg50[:])
                    for half, jt in enumerate((jp, jp + 1)):
                        kind = _tile_mask_kind(qb, jt)
                        psl = pt[:, half * BLK:(half + 1) * BLK]
                        if kind == "causal":
                            nc.gpsimd.affine_select(
                                out=psl, in_=psl,
                                compare_op=ALU.is_ge, fill=0.0,
                                base=qb * BLK - jt * 128,
                                pattern=[[1, BLK]], channel_multiplier=-1)
                        elif kind == "window":
                            nc.gpsimd.affine_select(
                                out=psl, in_=psl,
                                compare_op=ALU.is_ge, fill=0.0,
                                base=jt * 128 - qb * BLK + (WINDOW - 1),
                                pattern=[[-1, BLK]], channel_multiplier=1)
                    return pt

                def sum_pv(acc, pt, jp):
                    for half, jt in enumerate((jp, jp + 1)):
                        psl = pt[:, half * BLK:(half + 1) * BLK]
                        nc.tensor.matmul(acc[0:1, BLK:2 * BLK], ones_b[:],
                                         psl, start=(jt == lo), stop=(jt == hi))
                        nc.tensor.matmul(acc[:, 0:BLK],
                                         vnat[:, jt * 128:(jt + 1) * 128],
                                         psl, start=(jt == lo), stop=(jt == hi))

                for h in range(NH):
                    qh = qhat[:, h * BLK:(h + 1) * BLK]
                    acc = ps.tile([128, 2 * BLK], f32, tag="acc", bufs=1)
                    jps = list(range(lo, hi + 1, 2))
                    sp2 = scores(qh, jps[0])
                    for i, jp in enumerate(jps):
                        pt = act_chain(sp2, jp)
                        if i + 1 < len(jps):
                            if filler:
                                filler.pop(0)()
                            sp2 = scores(qh, jps[i + 1])
                        sum_pv(acc, pt, jp)
                    # normalize: oth = pv * broadcast(1/sums)
                    rs = work.tile([1, BLK], f32, tag="rq", bufs=2)
                    nc.vector.reciprocal_approx_fast(rs[:],
                                                     acc[0:1, BLK:2 * BLK])
                    rs_r = work.tile([1, BLK], f32r, tag="rs2", bufs=2)
                    with nc.allow_low_precision(reason="f32r bcast rhs"):
                        nc.vector.tensor_copy(rs_r[:], rs[:])
                    bco = ps.tile([128, 2 * BLK], f32, tag="uni", bufs=3)
                    nc.tensor.matmul(bco[:, 0:BLK], onesrow_r[:],
                                     rs_r[:], start=True, stop=True)
                    bco_s = work.tile([128, BLK], f32, tag="bc", bufs=2)
                    nc.vector.tensor_copy(bco_s[:], bco[:, 0:BLK])
                    oth = work.tile([128, BLK], bf16, tag="oth", bufs=8)
                    nc.vector.tensor_mul(oth[:], acc[:, 0:BLK], bco_s[:])
                    oth_tiles.append(oth)
                return oth_tiles

            # ---------------- main schedule ----------------
            blocks = [(b, qb) for b in range(B) for qb in range(NBLK)]
            pending = []        # o_proj items of the previous block
            for b, qb in blocks:
                tok0 = b * S + qb * BLK
                qp01, qp23, kvp = qkv_phase(b, qb)
                qhat = work.tile([128, NH * BLK], f32r, tag="qhat", bufs=1)
                norm_phase(b, qb, qp01, qp23, kvp, qhat, pending)
                oth_tiles = attn_phase(b, qb, qhat, pending)
                for it in pending:     # leftovers (early blocks)
                    it()
                pending = oproj_items(oth_tiles, tok0)
            for it in pending:
                it()
            if _debug:
                nc.sync.dma_start(dbg_khat[:], khat[:].bitcast(f32))
                nc.sync.dma_start(dbg_vnat[:], vnat[:])
                nc.sync.dma_start(dbg_qhat[:], qhat[:].bitcast(f32))

    nc.compile()
    return nc


def _host_inputs(x, wq, wk, wv, wo, q_norm_w, k_norm_w):
    """Build per-core input maps (host-side sharding + layout transforms)."""
    import ml_dtypes
    xT = np.ascontiguousarray(x.reshape(TOK, HID).T)  # [HID, TOK] shared
    xTb = xT.astype(ml_dtypes.bfloat16)

    inv_freq = 1.0 / (10000.0 ** (np.arange(0, HD, 2, dtype=np.float32) / HD))
    freqs = np.arange(S, dtype=np.float32)[:, None] * inv_freq  # [S, 64]
    c = np.cos(freqs).T.astype(np.float32)   # [64, S]
    sn = np.sin(freqs).T.astype(np.float32)
    cosT = np.concatenate([c, c], axis=0)                  # [cos;cos]
    sinT = np.concatenate([-sn, sn], axis=0)               # [-sin;sin]
    # fold the q/k RMSNorm weights into the rope tables: the rope output for
    # dim d mixes nrm[d] and nrm[(d+64)%128], both scaled by w[d] afterwards.
    qw = q_norm_w.reshape(128, 1).astype(np.float32)
    kw = k_norm_w.reshape(128, 1).astype(np.float32)
    qw_r = np.roll(qw, -64, axis=0)   # sin term mixes dim (d+64)%128
    kw_r = np.roll(kw, -64, axis=0)
    cosqT = np.ascontiguousarray(cosT * qw)
    sinqT = np.ascontiguousarray(sinT * qw_r)
    coskT = np.ascontiguousarray(cosT * kw)
    sinkT = np.ascontiguousarray(sinT * kw_r)

    def cat_tiles(wT):
        # [HID, width] -> [128, KTILES*width] (ktile k at cols k*width:...)
        return np.ascontiguousarray(
            np.concatenate([wT[k * 128:(k + 1) * 128, :]
                            for k in range(KTILES)], axis=1))

    in_maps = []
    for cidx in range(NCORES):
        wq_c = wq[cidx * QD:(cidx + 1) * QD, :].T          # [HID, 512]
        wk_c = wk[cidx * HD:(cidx + 1) * HD, :].T          # [HID, 128]
        wv_c = wv[cidx * HD:(cidx + 1) * HD, :].T          # [HID, 128]
        wo_c = wo[:, cidx * QD:(cidx + 1) * QD].T          # [512, HID]
        woT_cat = np.ascontiguousarray(
            np.concatenate([wo_c[kk * 128:(kk + 1) * 128, :]
                            for kk in range(NH)], axis=1))  # [128, 4*HID]
        in_maps.append({
            "xT": xTb,
            "wqT": cat_tiles(wq_c).astype(ml_dtypes.bfloat16),
            "wkT": cat_tiles(wk_c).astype(ml_dtypes.bfloat16),
            "wvT": cat_tiles(wv_c).astype(ml_dtypes.bfloat16),
            "woT": woT_cat.astype(ml_dtypes.bfloat16),
            "cosqT": cosqT.astype(ml_dtypes.bfloat16),
            "sinqT": sinqT.astype(ml_dtypes.bfloat16),
            "coskT": coskT.astype(ml_dtypes.bfloat16),
            "sinkT": sinkT.astype(ml_dtypes.bfloat16),
        })
    return in_maps


def kernel(x, wq, wk, wv, wo, q_norm_w, k_norm_w, _trace=False):
    from concourse import bass_utils

    x = np.asarray(x, np.float32)
    wq, wk, wv, wo = (np.asarray(a, np.float32) for a in (wq, wk, wv, wo))
    q_norm_w = np.asarray(q_norm_w, np.float32)
    k_norm_w = np.asarray(k_norm_w, np.float32)

    if "nc" not in _CACHE:
        _CACHE["nc"] = _build()
    nc = _CACHE["nc"]

    in_maps = _host_inputs(x, wq, wk, wv, wo, q_norm_w, k_norm_w)
    res = bass_utils.run_bass_kernel_spmd(
        nc, in_maps, core_ids=list(range(NCORES)), trace=_trace)
    _CACHE["last_result"] = res

    acc = np.zeros((HID, TOK), np.float32)
    for c in range(NCORES):
        acc += np.asarray(res.results[c]["outT"], np.float32)
    out = acc.T.reshape(B, S, HID)
    return out


# revision 32
# speedup vs baseline: 1.0392x; 1.0392x over previous
"""Sliding-window softcapped GQA attention, tensor-parallel across 8 NeuronCores.

Sharding (per spec hint): core c owns KV head c and Q heads 4c..4c+3.
Each core computes x->q/k/v proj, QK-RMSNorm, RoPE, windowed softcapped
attention, and its partial o_proj; host sums the 8 partial outputs.

v2: single fused pass per 512-token block. o_proj of block i-1 is emitted as
PE filler inside the norm/attention phases of block i (score-ahead pipelining
keeps the PE dense while the Act engine runs tanh/exp). Softmax/RMSNorm
reciprocals use the fast DVE approximation, partition broadcasts run on
GpSimd, the RoPE half-swap is a DVE stream_shuffle, and V is transposed by
the DMA XBAR. Weights and all latency-critical state stay resident in SBUF.
"""
import numpy as np

B, S, HID = 2, 2048, 4096
NQ, NK, HD = 32, 8, 128
WINDOW = 1024
SOFTCAP = 50.0
EPS = 1e-6
NCORES = 8
QD = NQ // NCORES * HD      # 512 q-dims per core
TOK = B * S                 # 4096 tokens
NBLK = 4                    # q-blocks of 512 per batch
BLK = 512
KTILES = HID // 128         # 32 k tiles over hidden
NH = NQ // NCORES           # 4 q heads per core

_CACHE = {}


def _window_jts(qb):
    lo = max(0, qb * BLK - (WINDOW - 1)) // 128
    hi = (qb * BLK + BLK - 1) // 128
    return lo, hi


def _tile_mask_kind(qb, jt):
    """None = fully allowed, 'causal' or 'window' = needs affine mask."""
    if jt * 128 + 127 > qb * BLK:
        return "causal"
    if jt * 128 < qb * BLK - BLK:
        return "window"
    return None


def _build(_debug=False):
    import concourse.bass as bass
    import concourse.mybir as mybir
    import concourse.tile as tile
    from concourse import bacc
    from concourse.masks import make_identity

    f32, f32r, bf16 = mybir.dt.float32, mybir.dt.float32r, mybir.dt.bfloat16
    AF = mybir.ActivationFunctionType
    ALU = mybir.AluOpType

    nc = bacc.Bacc("TRN2", target_bir_lowering=False, debug=False,
                   num_devices=NCORES)

    # ---- DRAM I/O (per-core shapes; same program on all cores) ----
    xT = nc.dram_tensor("xT", (HID, TOK), bf16, kind="ExternalInput").ap()
    wqT = nc.dram_tensor("wqT", (128, KTILES * QD), bf16,
                         kind="ExternalInput").ap()
    wkT = nc.dram_tensor("wkT", (128, KTILES * 128), bf16,
                         kind="ExternalInput").ap()
    wvT = nc.dram_tensor("wvT", (128, KTILES * 128), bf16,
                         kind="ExternalInput").ap()
    woT = nc.dram_tensor("woT", (128, NH * HID), bf16,
                         kind="ExternalInput").ap()
    cosqT = nc.dram_tensor("cosqT", (128, S), bf16, kind="ExternalInput").ap()
    sinqT = nc.dram_tensor("sinqT", (128, S), bf16, kind="ExternalInput").ap()
    coskT = nc.dram_tensor("coskT", (128, S), bf16, kind="ExternalInput").ap()
    sinkT = nc.dram_tensor("sinkT", (128, S), bf16, kind="ExternalInput").ap()
    outT = nc.dram_tensor("outT", (HID, TOK), bf16, kind="ExternalOutput").ap()
    if _debug:
        dbg_khat = nc.dram_tensor("dbg_khat", (128, S), f32,
                                  kind="ExternalOutput").ap()
        dbg_vnat = nc.dram_tensor("dbg_vnat", (128, S), bf16,
                                  kind="ExternalOutput").ap()
        dbg_qhat = nc.dram_tensor("dbg_qhat", (128, NH * BLK), f32,
                                  kind="ExternalOutput").ap()

    with tile.TileContext(nc) as tc:
        with tc.tile_pool(name="wts", bufs=1) as wts, \
             tc.tile_pool(name="stream", bufs=2) as stream, \
             tc.tile_pool(name="persist", bufs=1) as persist, \
             tc.tile_pool(name="work", bufs=2) as work, \
             tc.tile_pool(name="ps", bufs=1, space="PSUM") as ps:

            # ---- resident constants / weights ----
            wq_s = wts.tile([128, KTILES * QD], bf16)       # 32KB/p
            wk_s = wts.tile([128, KTILES * 128], bf16)      # 8KB/p
            wv_s = wts.tile([128, KTILES * 128], bf16)      # 8KB/p
            wo_s = wts.tile([128, NH * HID], bf16)          # 32KB/p
            for k in range(0, KTILES, 8):
                nc.sync.dma_start(wq_s[:, k * QD:(k + 8) * QD],
                                  wqT[:, k * QD:(k + 8) * QD])
            nc.sync.dma_start(wk_s[:], wkT[:])
            nc.sync.dma_start(wv_s[:], wvT[:])
            nc.sync.dma_start(wo_s[:], woT[:])
            cosq = wts.tile([128, S], bf16)
            sinq = wts.tile([128, S], bf16)
            cosk = wts.tile([128, S], bf16)
            sink = wts.tile([128, S], bf16)
            nc.sync.dma_start(cosq[:], cosqT[:])
            nc.sync.dma_start(sinq[:], sinqT[:])
            nc.sync.dma_start(cosk[:], coskT[:])
            nc.sync.dma_start(sink[:], sinkT[:])

            ones_b = wts.tile([128, 1], bf16)               # colsum lhsT
            nc.gpsimd.memset(ones_b[:], 1.0)
            onesrow_f = wts.tile([1, 128], f32)
            nc.gpsimd.memset(onesrow_f[:], 1.0)
            onesrow_r = wts.tile([1, 128], f32r)            # bcast lhsT
            nc.vector.tensor_copy(onesrow_r[:], onesrow_f[:])
            neg50 = wts.tile([128, 1], f32)
            nc.gpsimd.memset(neg50[:], -50.0)
            ident_f = wts.tile([128, 128], f32)
            make_identity(nc, ident_f[:])
            ident_b = wts.tile([128, 128], bf16)
            nc.vector.tensor_copy(ident_b[:], ident_f[:])
            # half-swap permutation: swap[i, j] = 1 iff j == (i+64) % 128
            swap_f = wts.tile([128, 128], f32)
            nc.gpsimd.memset(swap_f[:], 0.0)
            nc.gpsimd.affine_select(out=swap_f[:], in_=swap_f[:],
                                    compare_op=ALU.not_equal, fill=1.0,
                                    base=64, pattern=[[-1, 128]],
                                    channel_multiplier=1)
            nc.gpsimd.affine_select(out=swap_f[:], in_=swap_f[:],
                                    compare_op=ALU.not_equal, fill=1.0,
                                    base=-64, pattern=[[-1, 128]],
                                    channel_multiplier=1)
            swap_r = wts.tile([128, 128], f32r)
            nc.vector.tensor_copy(swap_r[:], swap_f[:])

            # persistent per-batch state (overwritten per batch)
            khat = persist.tile([128, S], f32r, tag="khat")
            vnat = persist.tile([128, S], bf16, tag="vnat")

            # ---------------- emission helpers ----------------

            def qkv_phase(b, qb):
                tok0 = b * S + qb * BLK
                qp01 = ps.tile([128, 2 * BLK], f32, tag="uni", bufs=3)
                qp23 = ps.tile([128, 2 * BLK], f32, tag="uni", bufs=3)
                kvp = ps.tile([128, 2 * BLK], f32, tag="uni", bufs=3)
                qsl = [qp01[:, 0:BLK], qp01[:, BLK:2 * BLK],
                       qp23[:, 0:BLK], qp23[:, BLK:2 * BLK]]
                ksl, vsl = kvp[:, 0:BLK], kvp[:, BLK:2 * BLK]
                for kk in range(KTILES):
                    sl = stream.tile([128, BLK], bf16, tag="slab", bufs=4)
                    nc.sync.dma_start(sl[:],
                                      xT[kk * 128:(kk + 1) * 128,
                                         tok0:tok0 + BLK])
                    if True:
                        xt = sl[:]
                        for m in range(NH):
                            nc.tensor.matmul(
                                qsl[m],
                                wq_s[:, kk * QD + m * 128:kk * QD + (m + 1) * 128],
                                xt, start=(kk == 0), stop=(kk == KTILES - 1))
                        nc.tensor.matmul(ksl, wk_s[:, kk * 128:(kk + 1) * 128],
                                         xt, start=(kk == 0), stop=(kk == KTILES - 1))
                        nc.tensor.matmul(vsl, wv_s[:, kk * 128:(kk + 1) * 128],
                                         xt, start=(kk == 0), stop=(kk == KTILES - 1))
                return qp01, qp23, kvp

            def norm_phase(b, qb, qp01, qp23, kvp, qhat, filler):
                """QK-RMSNorm + RoPE for 4 q heads + k; V -> vnat via PE.
                Copies the qkv psums to SBUF immediately so the uni psum
                slots free up for o_proj filler items of the prev block."""
                pos0 = qb * BLK
                # squares read the psums (Act), then the psums are copied to
                # SBUF (DVE) and released for the uni rotation.
                sq01 = work.tile([128, 2 * BLK], bf16, tag="sq", bufs=1)
                nc.scalar.activation(sq01[:], qp01[:], AF.Square)
                sq23 = work.tile([128, 2 * BLK], bf16, tag="sq2", bufs=1)
                nc.scalar.activation(sq23[:], qp23[:], AF.Square)
                sqk = work.tile([128, BLK], bf16, tag="sqk", bufs=1)
                nc.scalar.activation(sqk[:], kvp[:, 0:BLK], AF.Square)
                qraw = work.tile([128, 5 * BLK], f32r, tag="qraw", bufs=1)
                nc.vector.tensor_copy(qraw[:, 0:2 * BLK], qp01[:])
                nc.vector.tensor_copy(qraw[:, 2 * BLK:4 * BLK], qp23[:])
                nc.vector.tensor_copy(qraw[:, 4 * BLK:5 * BLK], kvp[:, 0:BLK])
                vT_s = work.tile([128, BLK], bf16, tag="vTs", bufs=1)
                nc.vector.tensor_copy(vT_s[:], kvp[:, BLK:2 * BLK])
                # V transpose to natural layout (PE, via acc rotation)
                vtr = ps.tile([128, BLK], bf16, tag="acc", bufs=1)
                for tt in range(4):
                    nc.tensor.transpose(vtr[:, tt * 128:(tt + 1) * 128],
                                        vT_s[:, tt * 128:(tt + 1) * 128],
                                        ident_b[:])
                nc.vector.tensor_copy(vnat[:, qb * BLK:(qb + 1) * BLK], vtr[:])
                for _ in range(5):
                    if filler:
                        filler.pop(0)()
                # column sums + rsqrt (folded scales):
                #   q: sqrt((1/ss)/2500) = rsqrt(ss)/50
                #   k: sqrt((1/ss)*HD)   = rsqrt(mean(k^2))
                sqs = [sq01[:, 0:BLK], sq01[:, BLK:2 * BLK],
                       sq23[:, 0:BLK], sq23[:, BLK:2 * BLK], sqk[:]]
                nrms = []
                groups = [((0, 1), 1.0 / (SOFTCAP * SOFTCAP)),
                          ((2, 3), 1.0 / (SOFTCAP * SOFTCAP)),
                          ((4,), float(HD))]
                for ms, scale in groups:
                    w = len(ms) * BLK
                    cst = ps.tile([1, 2 * BLK], f32, tag="acc", bufs=1)
                    for i, m in enumerate(ms):
                        nc.tensor.matmul(cst[0:1, i * BLK:(i + 1) * BLK],
                                         ones_b[:], sqs[m],
                                         start=True, stop=True)
                    rq = work.tile([1, 2 * BLK], f32, tag="rq", bufs=2)
                    nc.vector.reciprocal_approx_fast(rq[0:1, 0:w],
                                                     cst[0:1, 0:w])
                    rs2 = work.tile([1, 2 * BLK], f32r, tag="rs2", bufs=2)
                    with nc.allow_low_precision(reason="f32r bcast rhs"):
                        nc.scalar.activation(rs2[0:1, 0:w], rq[0:1, 0:w],
                                             AF.Sqrt, scale=scale)
                    bcq = ps.tile([128, 2 * BLK], f32, tag="uni", bufs=3)
                    for i, m in enumerate(ms):
                        nc.tensor.matmul(bcq[:, i * BLK:(i + 1) * BLK],
                                         onesrow_r[:],
                                         rs2[0:1, i * BLK:(i + 1) * BLK],
                                         start=True, stop=True)
                        nrm = work.tile([128, BLK], f32r, tag="nrm", bufs=5)
                        nc.vector.tensor_mul(
                            nrm[:], qraw[:, m * BLK:(m + 1) * BLK],
                            bcq[:, i * BLK:(i + 1) * BLK])
                        nrms.append((m, nrm))
                nrms = [nrms[0], nrms[4], nrms[1], nrms[2], nrms[3]]
                # rope per head (q0 first, then k, then q1..q3)
                for m, nrm in nrms:
                    rot = ps.tile([128, BLK], f32, tag="acc", bufs=1)
                    nc.tensor.matmul(rot[:], swap_r[:], nrm[:],
                                     start=True, stop=True)
                    if m < 4:
                        dst = qhat[:, m * BLK:(m + 1) * BLK]
                        cs_t, sn_t = cosq, sinq
                    else:
                        dst = khat[:, qb * BLK:(qb + 1) * BLK]
                        cs_t, sn_t = cosk, sink
                    nc.vector.tensor_mul(dst, nrm[:],
                                         cs_t[:, pos0:pos0 + BLK])
                    m2 = work.tile([128, BLK], f32, tag="m2", bufs=2)
                    nc.vector.tensor_mul(m2[:], rot[:],
                                         sn_t[:, pos0:pos0 + BLK])
                    nc.vector.tensor_add(dst, dst, m2[:])

            def oproj_items(oth_tiles, tok0):
                """o_proj work items for one finished block: 16 psum tiles,
                each covering two 128-row output feature tiles."""
                items = []
                for mp in range(16):
                    def item(m0=2 * mp):
                        op = ps.tile([128, 2 * BLK], f32, tag="uni", bufs=3)
                        for half in range(2):
                            m = m0 + half
                            for kk in range(NH):
                                nc.tensor.matmul(
                                    op[:, half * BLK:(half + 1) * BLK],
                                    wo_s[:, kk * HID + m * 128:kk * HID + (m + 1) * 128],
                                    oth_tiles[kk][:],
                                    start=(kk == 0), stop=(kk == NH - 1))
                        for half in range(2):
                            og = work.tile([128, BLK], bf16, tag="og", bufs=2)
                            nc.vector.tensor_copy(
                                og[:], op[:, half * BLK:(half + 1) * BLK])
                            m = m0 + half
                            nc.sync.dma_start(
                                outT[m * 128:(m + 1) * 128, tok0:tok0 + BLK],
                                og[:])
                    items.append(item)
                return items

            def attn_phase(b, qb, qhat, filler):
                """Attention for 4 heads with score-ahead pipelining; PE gaps
                are filled with o_proj items of the previous block."""
                lo, hi = _window_jts(qb)
                oth_tiles = []

                def scores(qh, jp):
                    sp2 = ps.tile([128, 2 * BLK], f32, tag="uni", bufs=3)
                    for half, jt in enumerate((jp, jp + 1)):
                        nc.tensor.matmul(
                            sp2[:, half * BLK:(half + 1) * BLK],
                            khat[:, jt * 128:(jt + 1) * 128],
                            qh, start=True, stop=True)
                    return sp2

                def act_chain(sp2, jp):
                    # softcap tanh in place on psum, exp to bf16, mask after
                    nc.scalar.activation(sp2[:], sp2[:], AF.Tanh)
                    pt = work.tile([128, 2 * BLK], bf16, tag="pt", bufs=2)
                    nc.scalar.activation(pt[:], sp2[:], AF.Exp,
                                         scale=SOFTCAP, bias=neg50[:])
                    for half, jt in enumerate((jp, jp + 1)):
                        kind = _tile_mask_kind(qb, jt)
                        psl = pt[:, half * BLK:(half + 1) * BLK]
                        if kind == "causal":
                            nc.gpsimd.affine_select(
                                out=psl, in_=psl,
                                compare_op=ALU.is_ge, fill=0.0,
                                base=qb * BLK - jt * 128,
                                pattern=[[1, BLK]], channel_multiplier=-1)
                        elif kind == "window":
                            nc.gpsimd.affine_select(
                                out=psl, in_=psl,
                                compare_op=ALU.is_ge, fill=0.0,
                                base=jt * 128 - qb * BLK + (WINDOW - 1),
                                pattern=[[-1, BLK]], channel_multiplier=1)
                    return pt

                def sum_pv(acc, pt, jp):
                    for half, jt in enumerate((jp, jp + 1)):
                        psl = pt[:, half * BLK:(half + 1) * BLK]
                        nc.tensor.matmul(acc[0:1, BLK:2 * BLK], ones_b[:],
                                         psl, start=(jt == lo), stop=(jt == hi))
                        nc.tensor.matmul(acc[:, 0:BLK],
                                         vnat[:, jt * 128:(jt + 1) * 128],
                                         psl, start=(jt == lo), stop=(jt == hi))

                for h in range(NH):
                    qh = qhat[:, h * BLK:(h + 1) * BLK]
                    acc = ps.tile([128, 2 * BLK], f32, tag="acc", bufs=1)
                    jps = list(range(lo, hi + 1, 2))
                    sp2 = scores(qh, jps[0])
                    for i, jp in enumerate(jps):
                        pt = act_chain(sp2, jp)
                        if i + 1 < len(jps):
                            if filler:
                                filler.pop(0)()
                            sp2 = scores(qh, jps[i + 1])
                        sum_pv(acc, pt, jp)
                    # normalize: oth = pv * broadcast(1/sums)
                    rs = work.tile([1, BLK], f32, tag="rq", bufs=2)
                    nc.vector.reciprocal_approx_fast(rs[:],
                                                     acc[0:1, BLK:2 * BLK])
                    rs_r = work.tile([1, BLK], f32r, tag="rs2", bufs=2)
                    with nc.allow_low_precision(reason="f32r bcast rhs"):
                        nc.vector.tensor_copy(rs_r[:], rs[:])
                    bco = ps.tile([128, 2 * BLK], f32, tag="uni", bufs=3)
                    nc.tensor.matmul(bco[:, 0:BLK], onesrow_r[:],
                                     rs_r[:], start=True, stop=True)
                    bco_s = work.tile([128, BLK], f32, tag="bc", bufs=2)
                    nc.vector.tensor_copy(bco_s[:], bco[:, 0:BLK])
                    oth = work.tile([128, BLK], bf16, tag="oth", bufs=8)
                    nc.vector.tensor_mul(oth[:], acc[:, 0:BLK], bco_s[:])
                    oth_tiles.append(oth)
                return oth_tiles

            # ---------------- main schedule ----------------
            blocks = [(b, qb) for b in range(B) for qb in range(NBLK)]
            pending = []        # o_proj items of the previous block
            for b, qb in blocks:
                tok0 = b * S + qb * BLK
                qp01, qp23, kvp = qkv_phase(b, qb)
                qhat = work.tile([128, NH * BLK], f32r, tag="qhat", bufs=1)
                norm_phase(b, qb, qp01, qp23, kvp, qhat, pending)
                oth_tiles = attn_phase(b, qb, qhat, pending)
                for it in pending:     # leftovers (early blocks)
                    it()
                pending = oproj_items(oth_tiles, tok0)
            for it in pending:
                it()
            if _debug:
                nc.sync.dma_start(dbg_khat[:], khat[:].bitcast(f32))
                nc.sync.dma_start(dbg_vnat[:], vnat[:])
                nc.sync.dma_start(dbg_qhat[:], qhat[:].bitcast(f32))

    nc.compile()
    return nc


def _host_inputs(x, wq, wk, wv, wo, q_norm_w, k_norm_w):
    """Build per-core input maps (host-side sharding + layout transforms)."""
    import ml_dtypes
    xT = np.ascontiguousarray(x.reshape(TOK, HID).T)  # [HID, TOK] shared
    xTb = xT.astype(ml_dtypes.bfloat16)

    inv_freq = 1.0 / (10000.0 ** (np.arange(0, HD, 2, dtype=np.float32) / HD))
    freqs = np.arange(S, dtype=np.float32)[:, None] * inv_freq  # [S, 64]
    c = np.cos(freqs).T.astype(np.float32)   # [64, S]
    sn = np.sin(freqs).T.astype(np.float32)
    cosT = np.concatenate([c, c], axis=0)                  # [cos;cos]
    sinT = np.concatenate([-sn, sn], axis=0)               # [-sin;sin]
    # fold the q/k RMSNorm weights into the rope tables: the rope output for
    # dim d mixes nrm[d] and nrm[(d+64)%128], both scaled by w[d] afterwards.
    qw = q_norm_w.reshape(128, 1).astype(np.float32)
    kw = k_norm_w.reshape(128, 1).astype(np.float32)
    qw_r = np.roll(qw, -64, axis=0)   # sin term mixes dim (d+64)%128
    kw_r = np.roll(kw, -64, axis=0)
    cosqT = np.ascontiguousarray(cosT * qw)
    sinqT = np.ascontiguousarray(sinT * qw_r)
    coskT = np.ascontiguousarray(cosT * kw)
    sinkT = np.ascontiguousarray(sinT * kw_r)

    def cat_tiles(wT):
        # [HID, width] -> [128, KTILES*width] (ktile k at cols k*width:...)
        return np.ascontiguousarray(
            np.concatenate([wT[k * 128:(k + 1) * 128, :]
                            for k in range(KTILES)], axis=1))

    in_maps = []
    for cidx in range(NCORES):
        wq_c = wq[cidx * QD:(cidx + 1) * QD, :].T          # [HID, 512]
        wk_c = wk[cidx * HD:(cidx + 1) * HD, :].T          # [HID, 128]
        wv_c = wv[cidx * HD:(cidx + 1) * HD, :].T          # [HID, 128]
        wo_c = wo[:, cidx * QD:(cidx + 1) * QD].T          # [512, HID]
        woT_cat = np.ascontiguousarray(
            np.concatenate([wo_c[kk * 128:(kk + 1) * 128, :]
                            for kk in range(NH)], axis=1))  # [128, 4*HID]
        in_maps.append({
            "xT": xTb,
            "wqT": cat_tiles(wq_c).astype(ml_dtypes.bfloat16),
            "wkT": cat_tiles(wk_c).astype(ml_dtypes.bfloat16),
            "wvT": cat_tiles(wv_c).astype(ml_dtypes.bfloat16),
            "woT": woT_cat.astype(ml_dtypes.bfloat16),
            "cosqT": cosqT.astype(ml_dtypes.bfloat16),
            "sinqT": sinqT.astype(ml_dtypes.bfloat16),
            "coskT": coskT.astype(ml_dtypes.bfloat16),
            "sinkT": sinkT.astype(ml_dtypes.bfloat16),
        })
    return in_maps


def kernel(x, wq, wk, wv, wo, q_norm_w, k_norm_w, _trace=False):
    from concourse import bass_utils

    x = np.asarray(x, np.float32)
    wq, wk, wv, wo = (np.asarray(a, np.float32) for a in (wq, wk, wv, wo))
    q_norm_w = np.asarray(q_norm_w, np.float32)
    k_norm_w = np.asarray(k_norm_w, np.float32)

    if "nc" not in _CACHE:
        _CACHE["nc"] = _build()
    nc = _CACHE["nc"]

    in_maps = _host_inputs(x, wq, wk, wv, wo, q_norm_w, k_norm_w)
    res = bass_utils.run_bass_kernel_spmd(
        nc, in_maps, core_ids=list(range(NCORES)), trace=_trace)
    _CACHE["last_result"] = res

    acc = np.zeros((HID, TOK), np.float32)
    for c in range(NCORES):
        acc += np.asarray(res.results[c]["outT"], np.float32)
    out = acc.T.reshape(B, S, HID)
    return out


# revision 33
# speedup vs baseline: 1.1496x; 1.1063x over previous
"""Sliding-window softcapped GQA attention, tensor-parallel across 8 NeuronCores.

Sharding (per spec hint): core c owns KV head c and Q heads 4c..4c+3.
Each core computes x->q/k/v proj, QK-RMSNorm, RoPE, windowed softcapped
attention, and its partial o_proj; host sums the 8 partial outputs.

v2: single fused pass per 512-token block. o_proj of block i-1 is emitted as
PE filler inside the norm/attention phases of block i (score-ahead pipelining
keeps the PE dense while the Act engine runs tanh/exp). Softmax/RMSNorm
reciprocals use the fast DVE approximation, partition broadcasts run on
GpSimd, the RoPE half-swap is a DVE stream_shuffle, and V is transposed by
the DMA XBAR. Weights and all latency-critical state stay resident in SBUF.
"""
import numpy as np

B, S, HID = 2, 2048, 4096
NQ, NK, HD = 32, 8, 128
WINDOW = 1024
SOFTCAP = 50.0
EPS = 1e-6
NCORES = 8
QD = NQ // NCORES * HD      # 512 q-dims per core
TOK = B * S                 # 4096 tokens
NBLK = 4                    # q-blocks of 512 per batch
BLK = 512
KTILES = HID // 128         # 32 k tiles over hidden
NH = NQ // NCORES           # 4 q heads per core

_CACHE = {}


def _window_jts(qb):
    lo = max(0, qb * BLK - (WINDOW - 1)) // 128
    hi = (qb * BLK + BLK - 1) // 128
    return lo, hi


def _tile_mask_kind(qb, jt):
    """None = fully allowed, 'causal' or 'window' = needs affine mask."""
    if jt * 128 + 127 > qb * BLK:
        return "causal"
    if jt * 128 < qb * BLK - BLK:
        return "window"
    return None


def _build(_debug=False):
    import concourse.bass as bass
    import concourse.mybir as mybir
    import concourse.tile as tile
    from concourse import bacc
    from concourse.masks import make_identity

    f32, f32r, bf16 = mybir.dt.float32, mybir.dt.float32r, mybir.dt.bfloat16
    AF = mybir.ActivationFunctionType
    ALU = mybir.AluOpType

    nc = bacc.Bacc("TRN2", target_bir_lowering=False, debug=False,
                   num_devices=NCORES)

    # ---- DRAM I/O (per-core shapes; same program on all cores) ----
    xT = nc.dram_tensor("xT", (HID, TOK), bf16, kind="ExternalInput").ap()
    wqT = nc.dram_tensor("wqT", (128, KTILES * QD), bf16,
                         kind="ExternalInput").ap()
    wkT = nc.dram_tensor("wkT", (128, KTILES * 128), bf16,
                         kind="ExternalInput").ap()
    wvT = nc.dram_tensor("wvT", (128, KTILES * 128), bf16,
                         kind="ExternalInput").ap()
    woT = nc.dram_tensor("woT", (128, NH * HID), bf16,
                         kind="ExternalInput").ap()
    cosqT = nc.dram_tensor("cosqT", (128, S), bf16, kind="ExternalInput").ap()
    sinqT = nc.dram_tensor("sinqT", (128, S), bf16, kind="ExternalInput").ap()
    coskT = nc.dram_tensor("coskT", (128, S), bf16, kind="ExternalInput").ap()
    sinkT = nc.dram_tensor("sinkT", (128, S), bf16, kind="ExternalInput").ap()
    outT = nc.dram_tensor("outT", (HID, TOK), bf16, kind="ExternalOutput").ap()
    if _debug:
        dbg_khat = nc.dram_tensor("dbg_khat", (128, S), f32,
                                  kind="ExternalOutput").ap()
        dbg_vnat = nc.dram_tensor("dbg_vnat", (128, S), bf16,
                                  kind="ExternalOutput").ap()
        dbg_qhat = nc.dram_tensor("dbg_qhat", (128, NH * BLK), f32,
                                  kind="ExternalOutput").ap()

    with tile.TileContext(nc) as tc:
        with tc.tile_pool(name="wts", bufs=1) as wts, \
             tc.tile_pool(name="stream", bufs=2) as stream, \
             tc.tile_pool(name="persist", bufs=1) as persist, \
             tc.tile_pool(name="work", bufs=2) as work, \
             tc.tile_pool(name="ps", bufs=1, space="PSUM") as ps:

            # ---- resident constants / weights ----
            wq_s = wts.tile([128, KTILES * QD], bf16)       # 32KB/p
            wk_s = wts.tile([128, KTILES * 128], bf16)      # 8KB/p
            wv_s = wts.tile([128, KTILES * 128], bf16)      # 8KB/p
            wo_s = wts.tile([128, NH * HID], bf16)          # 32KB/p
            for k in range(0, KTILES, 8):
                nc.sync.dma_start(wq_s[:, k * QD:(k + 8) * QD],
                                  wqT[:, k * QD:(k + 8) * QD])
            nc.sync.dma_start(wk_s[:], wkT[:])
            nc.sync.dma_start(wv_s[:], wvT[:])
            nc.sync.dma_start(wo_s[:], woT[:])
            cosq = wts.tile([128, S], bf16)
            sinq = wts.tile([128, S], bf16)
            cosk = wts.tile([128, S], bf16)
            sink = wts.tile([128, S], bf16)
            nc.sync.dma_start(cosq[:], cosqT[:])
            nc.sync.dma_start(sinq[:], sinqT[:])
            nc.sync.dma_start(cosk[:], coskT[:])
            nc.sync.dma_start(sink[:], sinkT[:])

            ones_b = wts.tile([128, 1], bf16)               # colsum lhsT
            nc.gpsimd.memset(ones_b[:], 1.0)
            onesrow_f = wts.tile([1, 128], f32)
            nc.gpsimd.memset(onesrow_f[:], 1.0)
            onesrow_r = wts.tile([1, 128], f32r)            # bcast lhsT
            nc.vector.tensor_copy(onesrow_r[:], onesrow_f[:])
            neg50 = wts.tile([128, 1], f32)
            nc.gpsimd.memset(neg50[:], -50.0)
            ident_f = wts.tile([128, 128], f32)
            make_identity(nc, ident_f[:])
            ident_b = wts.tile([128, 128], bf16)
            nc.vector.tensor_copy(ident_b[:], ident_f[:])
            # half-swap permutation: swap[i, j] = 1 iff j == (i+64) % 128
            swap_f = wts.tile([128, 128], f32)
            nc.gpsimd.memset(swap_f[:], 0.0)
            nc.gpsimd.affine_select(out=swap_f[:], in_=swap_f[:],
                                    compare_op=ALU.not_equal, fill=1.0,
                                    base=64, pattern=[[-1, 128]],
                                    channel_multiplier=1)
            nc.gpsimd.affine_select(out=swap_f[:], in_=swap_f[:],
                                    compare_op=ALU.not_equal, fill=1.0,
                                    base=-64, pattern=[[-1, 128]],
                                    channel_multiplier=1)
            swap_r = wts.tile([128, 128], f32r)
            nc.vector.tensor_copy(swap_r[:], swap_f[:])

            # persistent per-batch state (overwritten per batch)
            khat = persist.tile([128, S], f32r, tag="khat")
            vnat = persist.tile([128, S], bf16, tag="vnat")

            # ---------------- emission helpers ----------------

            def qkv_phase(b, qb):
                tok0 = b * S + qb * BLK
                qp01 = ps.tile([128, 2 * BLK], f32, tag="uni", bufs=3)
                qp23 = ps.tile([128, 2 * BLK], f32, tag="uni", bufs=3)
                kvp = ps.tile([128, 2 * BLK], f32, tag="uni", bufs=3)
                qsl = [qp01[:, 0:BLK], qp01[:, BLK:2 * BLK],
                       qp23[:, 0:BLK], qp23[:, BLK:2 * BLK]]
                ksl, vsl = kvp[:, 0:BLK], kvp[:, BLK:2 * BLK]
                for kk in range(KTILES):
                    sl = stream.tile([128, BLK], bf16, tag="slab", bufs=4)
                    nc.sync.dma_start(sl[:],
                                      xT[kk * 128:(kk + 1) * 128,
                                         tok0:tok0 + BLK])
                    if True:
                        xt = sl[:]
                        for m in range(NH):
                            nc.tensor.matmul(
                                qsl[m],
                                wq_s[:, kk * QD + m * 128:kk * QD + (m + 1) * 128],
                                xt, start=(kk == 0), stop=(kk == KTILES - 1))
                        nc.tensor.matmul(ksl, wk_s[:, kk * 128:(kk + 1) * 128],
                                         xt, start=(kk == 0), stop=(kk == KTILES - 1))
                        nc.tensor.matmul(vsl, wv_s[:, kk * 128:(kk + 1) * 128],
                                         xt, start=(kk == 0), stop=(kk == KTILES - 1))
                return qp01, qp23, kvp

            def norm_phase(b, qb, qp01, qp23, kvp, qhat, filler):
                """QK-RMSNorm + RoPE for 4 q heads + k; V -> vnat via PE.
                Copies the qkv psums to SBUF immediately so the uni psum
                slots free up for o_proj filler items of the prev block."""
                pos0 = qb * BLK
                # squares read the psums (Act), then the psums are copied to
                # SBUF (DVE) and released for the uni rotation.
                sq01 = work.tile([128, 2 * BLK], bf16, tag="sq", bufs=1)
                nc.scalar.activation(sq01[:], qp01[:], AF.Square)
                sq23 = work.tile([128, 2 * BLK], bf16, tag="sq2", bufs=1)
                nc.scalar.activation(sq23[:], qp23[:], AF.Square)
                sqk = work.tile([128, BLK], bf16, tag="sqk", bufs=1)
                nc.scalar.activation(sqk[:], kvp[:, 0:BLK], AF.Square)
                qraw = work.tile([128, 5 * BLK], f32r, tag="qraw", bufs=1)
                nc.vector.tensor_copy(qraw[:, 0:2 * BLK], qp01[:])
                nc.vector.tensor_copy(qraw[:, 2 * BLK:4 * BLK], qp23[:])
                nc.vector.tensor_copy(qraw[:, 4 * BLK:5 * BLK], kvp[:, 0:BLK])
                vT_s = work.tile([128, BLK], bf16, tag="vTs", bufs=1)
                nc.vector.tensor_copy(vT_s[:], kvp[:, BLK:2 * BLK])
                # V transpose to natural layout (PE, via acc rotation)
                vtr = ps.tile([128, BLK], bf16, tag="acc", bufs=1)
                for tt in range(4):
                    nc.tensor.transpose(vtr[:, tt * 128:(tt + 1) * 128],
                                        vT_s[:, tt * 128:(tt + 1) * 128],
                                        ident_b[:])
                nc.vector.tensor_copy(vnat[:, qb * BLK:(qb + 1) * BLK], vtr[:])
                for _ in range(5):
                    if filler:
                        filler.pop(0)()
                # column sums + rsqrt (folded scales):
                #   q: sqrt((1/ss)/2500) = rsqrt(ss)/50
                #   k: sqrt((1/ss)*HD)   = rsqrt(mean(k^2))
                sqs = [sq01[:, 0:BLK], sq01[:, BLK:2 * BLK],
                       sq23[:, 0:BLK], sq23[:, BLK:2 * BLK], sqk[:]]
                nrms = []
                groups = [((0, 1), 1.0 / (SOFTCAP * SOFTCAP)),
                          ((2, 3), 1.0 / (SOFTCAP * SOFTCAP)),
                          ((4,), float(HD))]
                for ms, scale in groups:
                    w = len(ms) * BLK
                    cst = ps.tile([1, 2 * BLK], f32, tag="acc", bufs=1)
                    for i, m in enumerate(ms):
                        nc.tensor.matmul(cst[0:1, i * BLK:(i + 1) * BLK],
                                         ones_b[:], sqs[m],
                                         start=True, stop=True)
                    rq = work.tile([1, 2 * BLK], f32, tag="rq", bufs=2)
                    nc.vector.reciprocal_approx_fast(rq[0:1, 0:w],
                                                     cst[0:1, 0:w])
                    rs2 = work.tile([1, 2 * BLK], f32, tag="rs2", bufs=2)
                    nc.scalar.activation(rs2[0:1, 0:w], rq[0:1, 0:w],
                                         AF.Sqrt, scale=scale)
                    for i, m in enumerate(ms):
                        bcq = work.tile([128, BLK], f32, tag="bc", bufs=2)
                        nc.gpsimd.partition_broadcast(
                            bcq[:], rs2[0:1, i * BLK:(i + 1) * BLK])
                        nrm = work.tile([128, BLK], f32r, tag="nrm", bufs=5)
                        with nc.allow_low_precision(reason="rope operand"):
                            nc.vector.tensor_mul(
                                nrm[:], qraw[:, m * BLK:(m + 1) * BLK], bcq[:])
                        nrms.append((m, nrm))
                nrms = [nrms[0], nrms[4], nrms[1], nrms[2], nrms[3]]
                # rope per head (q0 first, then k, then q1..q3); the half-swap
                # runs as two partition-offset SBUF DMAs on the pool queue
                for m, nrm in nrms:
                    rot = work.tile([128, BLK], f32r, tag="rot", bufs=2)
                    nc.gpsimd.dma_start(rot[0:64, :], nrm[64:128, :])
                    nc.gpsimd.dma_start(rot[64:128, :], nrm[0:64, :])
                    if m < 4:
                        dst = qhat[:, m * BLK:(m + 1) * BLK]
                        cs_t, sn_t = cosq, sinq
                    else:
                        dst = khat[:, qb * BLK:(qb + 1) * BLK]
                        cs_t, sn_t = cosk, sink
                    nc.vector.tensor_mul(dst, nrm[:],
                                         cs_t[:, pos0:pos0 + BLK])
                    m2 = work.tile([128, BLK], f32, tag="m2", bufs=2)
                    nc.vector.tensor_mul(m2[:], rot[:],
                                         sn_t[:, pos0:pos0 + BLK])
                    nc.vector.tensor_add(dst, dst, m2[:])

            def oproj_items(oth_tiles, tok0):
                """o_proj work items for one finished block: 16 psum tiles,
                each covering two 128-row output feature tiles."""
                items = []
                for mp in range(16):
                    def item(m0=2 * mp):
                        op = ps.tile([128, 2 * BLK], f32, tag="uni", bufs=3)
                        for half in range(2):
                            m = m0 + half
                            for kk in range(NH):
                                nc.tensor.matmul(
                                    op[:, half * BLK:(half + 1) * BLK],
                                    wo_s[:, kk * HID + m * 128:kk * HID + (m + 1) * 128],
                                    oth_tiles[kk][:],
                                    start=(kk == 0), stop=(kk == NH - 1))
                        for half in range(2):
                            og = work.tile([128, BLK], bf16, tag="og", bufs=2)
                            nc.vector.tensor_copy(
                                og[:], op[:, half * BLK:(half + 1) * BLK])
                            m = m0 + half
                            nc.sync.dma_start(
                                outT[m * 128:(m + 1) * 128, tok0:tok0 + BLK],
                                og[:])
                    items.append(item)
                return items

            def attn_phase(b, qb, qhat, filler):
                """Attention for 4 heads with score-ahead pipelining; PE gaps
                are filled with o_proj items of the previous block."""
                lo, hi = _window_jts(qb)
                oth_tiles = []

                def scores(qh, jp):
                    sp2 = ps.tile([128, 2 * BLK], f32, tag="uni", bufs=3)
                    for half, jt in enumerate((jp, jp + 1)):
                        nc.tensor.matmul(
                            sp2[:, half * BLK:(half + 1) * BLK],
                            khat[:, jt * 128:(jt + 1) * 128],
                            qh, start=True, stop=True)
                    return sp2

                def act_chain(sp2, jp):
                    # softcap tanh in place on psum, exp to bf16, mask after
                    nc.scalar.activation(sp2[:], sp2[:], AF.Tanh)
                    pt = work.tile([128, 2 * BLK], bf16, tag="pt", bufs=2)
                    nc.scalar.activation(pt[:], sp2[:], AF.Exp,
                                         scale=SOFTCAP, bias=neg50[:])
                    for half, jt in enumerate((jp, jp + 1)):
                        kind = _tile_mask_kind(qb, jt)
                        psl = pt[:, half * BLK:(half + 1) * BLK]
                        if kind == "causal":
                            nc.gpsimd.affine_select(
                                out=psl, in_=psl,
                                compare_op=ALU.is_ge, fill=0.0,
                                base=qb * BLK - jt * 128,
                                pattern=[[1, BLK]], channel_multiplier=-1)
                        elif kind == "window":
                            nc.gpsimd.affine_select(
                                out=psl, in_=psl,
                                compare_op=ALU.is_ge, fill=0.0,
                                base=jt * 128 - qb * BLK + (WINDOW - 1),
                                pattern=[[-1, BLK]], channel_multiplier=1)
                    return pt

                def sum_pv(acc, pt, jp):
                    for half, jt in enumerate((jp, jp + 1)):
                        psl = pt[:, half * BLK:(half + 1) * BLK]
                        nc.tensor.matmul(acc[0:1, BLK:2 * BLK], ones_b[:],
                                         psl, start=(jt == lo), stop=(jt == hi))
                        nc.tensor.matmul(acc[:, 0:BLK],
                                         vnat[:, jt * 128:(jt + 1) * 128],
                                         psl, start=(jt == lo), stop=(jt == hi))

                for h in range(NH):
                    qh = qhat[:, h * BLK:(h + 1) * BLK]
                    acc = ps.tile([128, 2 * BLK], f32, tag="acc", bufs=1)
                    jps = list(range(lo, hi + 1, 2))
                    sp2 = scores(qh, jps[0])
                    for i, jp in enumerate(jps):
                        pt = act_chain(sp2, jp)
                        if i + 1 < len(jps):
                            if filler:
                                filler.pop(0)()
                            sp2 = scores(qh, jps[i + 1])
                        sum_pv(acc, pt, jp)
                    # normalize: oth = pv * broadcast(1/sums)
                    rs = work.tile([1, BLK], f32, tag="rq", bufs=2)
                    nc.vector.reciprocal_approx_fast(rs[:],
                                                     acc[0:1, BLK:2 * BLK])
                    bco = work.tile([128, BLK], f32, tag="bc", bufs=2)
                    nc.gpsimd.partition_broadcast(bco[:], rs[:])
                    oth = work.tile([128, BLK], bf16, tag="oth", bufs=8)
                    nc.vector.tensor_mul(oth[:], acc[:, 0:BLK], bco[:])
                    oth_tiles.append(oth)
                return oth_tiles

            # ---------------- main schedule ----------------
            blocks = [(b, qb) for b in range(B) for qb in range(NBLK)]
            pending = []        # o_proj items of the previous block
            for b, qb in blocks:
                tok0 = b * S + qb * BLK
                qp01, qp23, kvp = qkv_phase(b, qb)
                qhat = work.tile([128, NH * BLK], f32r, tag="qhat", bufs=1)
                norm_phase(b, qb, qp01, qp23, kvp, qhat, pending)
                oth_tiles = attn_phase(b, qb, qhat, pending)
                for it in pending:     # leftovers (early blocks)
                    it()
                pending = oproj_items(oth_tiles, tok0)
            for it in pending:
                it()
            if _debug:
                nc.sync.dma_start(dbg_khat[:], khat[:].bitcast(f32))
                nc.sync.dma_start(dbg_vnat[:], vnat[:])
                nc.sync.dma_start(dbg_qhat[:], qhat[:].bitcast(f32))

    nc.compile()
    return nc


def _host_inputs(x, wq, wk, wv, wo, q_norm_w, k_norm_w):
    """Build per-core input maps (host-side sharding + layout transforms)."""
    import ml_dtypes
    xT = np.ascontiguousarray(x.reshape(TOK, HID).T)  # [HID, TOK] shared
    xTb = xT.astype(ml_dtypes.bfloat16)

    inv_freq = 1.0 / (10000.0 ** (np.arange(0, HD, 2, dtype=np.float32) / HD))
    freqs = np.arange(S, dtype=np.float32)[:, None] * inv_freq  # [S, 64]
    c = np.cos(freqs).T.astype(np.float32)   # [64, S]
    sn = np.sin(freqs).T.astype(np.float32)
    cosT = np.concatenate([c, c], axis=0)                  # [cos;cos]
    sinT = np.concatenate([-sn, sn], axis=0)               # [-sin;sin]
    # fold the q/k RMSNorm weights into the rope tables: the rope output for
    # dim d mixes nrm[d] and nrm[(d+64)%128], both scaled by w[d] afterwards.
    qw = q_norm_w.reshape(128, 1).astype(np.float32)
    kw = k_norm_w.reshape(128, 1).astype(np.float32)
    qw_r = np.roll(qw, -64, axis=0)   # sin term mixes dim (d+64)%128
    kw_r = np.roll(kw, -64, axis=0)
    cosqT = np.ascontiguousarray(cosT * qw)
    sinqT = np.ascontiguousarray(sinT * qw_r)
    coskT = np.ascontiguousarray(cosT * kw)
    sinkT = np.ascontiguousarray(sinT * kw_r)

    def cat_tiles(wT):
        # [HID, width] -> [128, KTILES*width] (ktile k at cols k*width:...)
        return np.ascontiguousarray(
            np.concatenate([wT[k * 128:(k + 1) * 128, :]
                            for k in range(KTILES)], axis=1))

    in_maps = []
    for cidx in range(NCORES):
        wq_c = wq[cidx * QD:(cidx + 1) * QD, :].T          # [HID, 512]
        wk_c = wk[cidx * HD:(cidx + 1) * HD, :].T          # [HID, 128]
        wv_c = wv[cidx * HD:(cidx + 1) * HD, :].T          # [HID, 128]
        wo_c = wo[:, cidx * QD:(cidx + 1) * QD].T          # [512, HID]
        woT_cat = np.ascontiguousarray(
            np.concatenate([wo_c[kk * 128:(kk + 1) * 128, :]
                            for kk in range(NH)], axis=1))  # [128, 4*HID]
        in_maps.append({
            "xT": xTb,
            "wqT": cat_tiles(wq_c).astype(ml_dtypes.bfloat16),
            "wkT": cat_tiles(wk_c).astype(ml_dtypes.bfloat16),
            "wvT": cat_tiles(wv_c).astype(ml_dtypes.bfloat16),
            "woT": woT_cat.astype(ml_dtypes.bfloat16),
            "cosqT": cosqT.astype(ml_dtypes.bfloat16),
            "sinqT": sinqT.astype(ml_dtypes.bfloat16),
            "coskT": coskT.astype(ml_dtypes.bfloat16),
            "sinkT": sinkT.astype(ml_dtypes.bfloat16),
        })
    return in_maps


def kernel(x, wq, wk, wv, wo, q_norm_w, k_norm_w, _trace=False):
    from concourse import bass_utils

    x = np.asarray(x, np.float32)
    wq, wk, wv, wo = (np.asarray(a, np.float32) for a in (wq, wk, wv, wo))
    q_norm_w = np.asarray(q_norm_w, np.float32)
    k_norm_w = np.asarray(k_norm_w, np.float32)

    if "nc" not in _CACHE:
        _CACHE["nc"] = _build()
    nc = _CACHE["nc"]

    in_maps = _host_inputs(x, wq, wk, wv, wo, q_norm_w, k_norm_w)
    res = bass_utils.run_bass_kernel_spmd(
        nc, in_maps, core_ids=list(range(NCORES)), trace=_trace)
    _CACHE["last_result"] = res

    acc = np.zeros((HID, TOK), np.float32)
    for c in range(NCORES):
        acc += np.asarray(res.results[c]["outT"], np.float32)
    out = acc.T.reshape(B, S, HID)
    return out
